# revision 1
# baseline (speedup 1.0000x reference)
"""Bahdanau attention kernel for Trainium2 (8 NeuronCores, SPMD data-parallel).

Reference computation (per batch b):
    f_proj = features[b] @ W1_w + W1_b            # [T, U]
    h_proj = hidden[b] @ W2_w + W2_b              # [U]
    score  = tanh(f_proj + h_proj) @ V_w + V_b    # [T]
    attn   = softmax(score)                       # [T]
    context[b] = sum_t attn[t] * features[b, t]   # [D]

Sharding: data-parallel over batch (64 batches / 8 cores = 8 per core),
weights replicated.

Per-core dataflow (everything fp32; matmuls optionally in float32r):
  - F tiles [128(t), 512(d)] are DMA'd in natively, PE-transposed
    (via identity matmul) into F^T [128(d), t] for the main matmul.
  - main matmul computes f_proj TRANSPOSED: [u(part), t(free)] =
    W1_chunk^T @ F^T, so the (W1_b + h_proj) bias is a per-partition
    scalar that fuses into the ACT Tanh instruction.
  - score^T [1, t] = V^T @ tanh via M=1 matmuls; ACT Exp with fused
    accum_out produces both e = exp(score + V_b) and its running sum.
    No max-subtraction: |score| <= ||V||_1 + |V_b| ~ 18, safe in fp32.
  - e rows are transposed to columns with tiny 1x1-identity matmuls;
    context accumulates as e_col^T @ F_native; final scale by 1/sum(e).
"""

import sys

for _p in ("/opt/trn_rl_repo", "/opt/pypackages"):
    if _p not in sys.path:
        sys.path.insert(0, _p)

import numpy as np

B, T, D, U = 64, 2048, 512, 512
NCORES = 8
BPC = B // NCORES          # batches per core
PART = 128
DC = D // PART             # 4 contraction chunks
UC = U // PART             # 4 u chunks
TCHUNK = 512               # t columns processed per main-matmul group
TILES_PER_CHUNK = TCHUNK // PART          # 4
NCHUNKS = (BPC * T) // TCHUNK             # 32
CHUNKS_PER_BATCH = T // TCHUNK            # 4

MM_DT_NAME = "float32r"    # dtype tag for matmul operands


_BUILD_CACHE = {}


def build_bass(mm_dt_name=MM_DT_NAME):
    """Build + compile the per-core Bass program (same on all cores)."""
    if mm_dt_name in _BUILD_CACHE:
        return _BUILD_CACHE[mm_dt_name]

    import concourse.mybir as mybir
    import concourse.tile as tile
    from concourse import bacc
    from concourse.bass import ts
    from concourse.masks import make_identity

    f32 = mybir.dt.float32
    mdt = getattr(mybir.dt, mm_dt_name)
    ACT = mybir.ActivationFunctionType
    AX = mybir.AxisListType

    nc = bacc.Bacc("TRN2", target_bir_lowering=False, debug=False)

    feat = nc.dram_tensor("features", [BPC, T, D], mdt, kind="ExternalInput")
    hid = nc.dram_tensor("hidden", [BPC, D], mdt, kind="ExternalInput")
    w1 = nc.dram_tensor("W1_w", [D, U], mdt, kind="ExternalInput")
    b1 = nc.dram_tensor("W1_b", [U], f32, kind="ExternalInput")
    w2 = nc.dram_tensor("W2_w", [D, U], mdt, kind="ExternalInput")
    b2 = nc.dram_tensor("W2_b", [U], f32, kind="ExternalInput")
    vw = nc.dram_tensor("V_w", [U, 1], mdt, kind="ExternalInput")
    vb = nc.dram_tensor("V_b", [1], f32, kind="ExternalInput")
    out = nc.dram_tensor("context", [BPC, D], f32, kind="ExternalOutput")

    with tile.TileContext(nc) as tc:
        with (
            tc.tile_pool(name="consts", bufs=1) as consts,
            tc.tile_pool(name="fpool", bufs=16) as fpool,
            tc.tile_pool(name="ftb", bufs=3) as ftb,
            tc.tile_pool(name="tanh", bufs=3) as tanhp,
            tc.tile_pool(name="small", bufs=3) as small,
            tc.tile_pool(name="outp", bufs=2) as outp,
            tc.tile_pool(name="ps_mm", bufs=3, space="PSUM") as ps_mm,
            tc.tile_pool(name="ps_t", bufs=3, space="PSUM") as ps_t,
            tc.tile_pool(name="ps_s", bufs=1, space="PSUM") as ps_s,
            tc.tile_pool(name="ps_c", bufs=1, space="PSUM") as ps_c,
        ):
            # ---------------- constants / setup ----------------
            ident_f32 = consts.tile([PART, PART], f32)
            make_identity(nc, ident_f32)
            ident = consts.tile([PART, PART], mdt)
            nc.vector.tensor_copy(ident, ident_f32)

            # preload the first two chunks' F tiles so the PE can start on
            # their transposes before the (large) weight DMAs complete
            preloaded = {}
            for pch in (0, 1):
                pb = pch // CHUNKS_PER_BATCH
                pt0 = (pch % CHUNKS_PER_BATCH) * TCHUNK
                tiles = []
                for j in range(TILES_PER_CHUNK):
                    f_pre = fpool.tile([PART, D], mdt, tag="F", name=f"f_pre_{pch}_{j}")
                    eng = nc.sync if j % 2 == 0 else nc.scalar
                    eng.dma_start(
                        out=f_pre,
                        in_=feat.ap()[pb, pt0 + j * PART : pt0 + (j + 1) * PART, :],
                    )
                    tiles.append(f_pre)
                preloaded[pch] = tiles

            w1_sb = consts.tile([PART, DC, U], mdt)
            w2_sb = consts.tile([PART, DC, U], mdt)
            for dc in range(DC):
                eng = nc.sync if dc % 2 == 0 else nc.scalar
                eng.dma_start(
                    out=w2_sb[:, dc, :], in_=w2.ap()[dc * PART : (dc + 1) * PART, :]
                )
            for dc in range(DC):
                eng = nc.sync if dc % 2 == 0 else nc.scalar
                eng.dma_start(
                    out=w1_sb[:, dc, :], in_=w1.ap()[dc * PART : (dc + 1) * PART, :]
                )
            v_sb = consts.tile([PART, UC], mdt)
            nc.sync.dma_start(out=v_sb, in_=vw.ap().rearrange("(c p) one -> p (c one)", p=PART))
            vb_sb = consts.tile([1, 1], f32)
            nc.sync.dma_start(out=vb_sb, in_=vb.ap().rearrange("(one x) -> one x", one=1))

            # W1_b + W2_b as per-partition columns [128, uc]
            b1_sb = consts.tile([PART, UC], f32)
            nc.sync.dma_start(out=b1_sb, in_=b1.ap().rearrange("(c p) -> p c", p=PART))
            b2_sb = consts.tile([PART, UC], f32)
            nc.sync.dma_start(out=b2_sb, in_=b2.ap().rearrange("(c p) -> p c", p=PART))
            b12_sb = consts.tile([PART, UC], f32)
            nc.vector.tensor_add(b12_sb, b1_sb, b2_sb)

            # hidden [BPC, D] -> hiddenT [128(d), dc, BPC]
            hid_sb = consts.tile([BPC, D], mdt)
            nc.sync.dma_start(out=hid_sb, in_=hid.ap())
            hidT_sb = consts.tile([PART, DC, BPC], mdt)
            bias_cols = consts.tile([PART, UC, BPC], f32)

            def emit_setup():
                # emitted after chunk 0's transposes so the PE isn't blocked
                # on the weight/hidden DMAs at kernel start
                for dc in range(DC):
                    ps_h = ps_t.tile([PART, TCHUNK], mdt, tag="T", name="ps_h")
                    nc.tensor.transpose(ps_h[:, 0:BPC], hid_sb[:, ts(dc, PART)], ident[0:BPC, 0:BPC])
                    nc.vector.tensor_copy(hidT_sb[:, dc, :], ps_h[:, 0:BPC])
                # h_projT[u, b] = sum_dc W2[dc]^T @ hiddenT[dc]  (+W2_b+W1_b)
                for uc in range(UC):
                    ps_h = ps_t.tile([PART, TCHUNK], f32, tag="T", name="ps_h2")
                    for dc in range(DC):
                        nc.tensor.matmul(
                            ps_h[:, 0:BPC],
                            w2_sb[:, dc, ts(uc, PART)],
                            hidT_sb[:, dc, :],
                            start=(dc == 0),
                            stop=(dc == DC - 1),
                        )
                    nc.vector.tensor_scalar_add(
                        bias_cols[:, uc, :], ps_h[:, 0:BPC], b12_sb[:, uc : uc + 1]
                    )

            # ---------------- main loop (epilogue deferred one chunk) ----------------
            prev = None          # chunk state awaiting its score/context stage
            batch_state = {}     # per-batch psum_ctx / running-sum tiles

            def emit_scores(st):
                b, cib = st["b"], st["cib"]
                if cib == 0:
                    batch_state["ps_ctx"] = ps_c.tile([1, D], f32, tag="ctx", name="ps_ctx")
                    batch_state["s_sb"] = small.tile([1, CHUNKS_PER_BATCH], f32, tag="ssum", name="s_sb")
                s_sb = batch_state["s_sb"]

                # score^T [1, t] = V^T @ tanh
                ps_sc = ps_s.tile([1, TCHUNK], f32, tag="score")
                for uc in range(UC):
                    nc.tensor.matmul(
                        ps_sc,
                        v_sb[:, uc : uc + 1],
                        st["tanh"][:, uc, :],
                        start=(uc == 0),
                        stop=(uc == UC - 1),
                    )
                # e = exp(score + V_b); row sum via explicit DVE reduction
                # (avoids relying on the split ACTIVATION_READ_ACCUMULATOR op)
                e_sb = small.tile([1, TCHUNK], mdt, tag="e_sb")
                nc.scalar.activation(e_sb, ps_sc, ACT.Exp, bias=vb_sb)
                nc.vector.reduce_sum(s_sb[:, cib : cib + 1], e_sb, axis=AX.X)
                st["e_sb"] = e_sb

            def emit_context(st):
                b, cib = st["b"], st["cib"]
                ps_ctx = batch_state["ps_ctx"]
                s_sb = batch_state["s_sb"]
                e_sb = st["e_sb"]
                # transpose e row -> columns via [1,0]-identity-row matmuls
                # (pairs of output columns keep fp32r ISA patterns even)
                ps_e = ps_s.tile([PART, 2 * TILES_PER_CHUNK], f32, tag="score", name="ps_e")
                for j in range(TILES_PER_CHUNK):
                    nc.tensor.matmul(
                        ps_e[:, 2 * j : 2 * j + 2],
                        e_sb[0:1, ts(j, PART)],
                        ident[0:1, 0:2],
                        start=True,
                        stop=True,
                    )
                e_col = small.tile([PART, TILES_PER_CHUNK], mdt, tag="e_col")
                nc.vector.tensor_copy(
                    e_col, ps_e.rearrange("p (j two) -> p two j", two=2)[:, 0, :]
                )
                # context accumulation: ps_ctx [1, D] += e_col_j^T @ F_j
                for j in range(TILES_PER_CHUNK):
                    nc.tensor.matmul(
                        ps_ctx,
                        e_col[:, j : j + 1],
                        st["f_tiles"][j],
                        start=(cib == 0 and j == 0),
                        stop=(cib == CHUNKS_PER_BATCH - 1 and j == TILES_PER_CHUNK - 1),
                    )
                if cib == CHUNKS_PER_BATCH - 1:
                    ssum = small.tile([1, 1], f32, tag="ssum1")
                    nc.vector.reduce_sum(ssum, s_sb, axis=AX.X)
                    rec = small.tile([1, 1], f32, tag="rec")
                    nc.vector.reciprocal(rec, ssum)
                    ctx_sb = outp.tile([1, D], f32, tag="ctx_sb")
                    nc.vector.tensor_scalar_mul(ctx_sb, ps_ctx, rec)
                    nc.sync.dma_start(out=out.ap()[b : b + 1, :], in_=ctx_sb)

            s1_state = {}

            def emit_s1(c):
                sb_ = c // CHUNKS_PER_BATCH
                st0 = (c % CHUNKS_PER_BATCH) * TCHUNK
                f_tiles = []
                ftile_big = ftb.tile([PART, DC, TCHUNK], mdt, tag="FT", name="ftile_big")
                for j in range(TILES_PER_CHUNK):
                    if c in preloaded:
                        f_ij = preloaded[c][j]
                    else:
                        f_ij = fpool.tile([PART, D], mdt, tag="F", name="f_ij")
                        dma_eng = nc.sync if j % 2 == 0 else nc.scalar
                        dma_eng.dma_start(
                            out=f_ij,
                            in_=feat.ap()[sb_, st0 + j * PART : st0 + (j + 1) * PART, :],
                        )
                    f_tiles.append(f_ij)
                    ps_tr = ps_t.tile([PART, TCHUNK], mdt, tag="T", name="ps_tr")
                    for dc in range(DC):
                        nc.tensor.transpose(
                            ps_tr[:, ts(dc, PART)], f_ij[:, ts(dc, PART)], ident
                        )
                    nc.vector.tensor_copy(
                        ftile_big[:, :, ts(j, PART)],
                        ps_tr.rearrange("p (c t) -> p c t", c=DC),
                    )
                s1_state[c] = (f_tiles, ftile_big)

            emit_s1(0)

            for chunk in range(NCHUNKS + 1):
                # V-dot + exp of the previous chunk lead this chunk
                if prev is not None:
                    emit_scores(prev)

                # transpose stage one chunk ahead, so this chunk's mains
                # never wait on the F^T copies
                if chunk + 1 < NCHUNKS:
                    emit_s1(chunk + 1)

                if chunk == 0:
                    emit_setup()

                if chunk < NCHUNKS:
                    b = chunk // CHUNKS_PER_BATCH
                    cib = chunk % CHUNKS_PER_BATCH
                    f_tiles, ftile_big = s1_state.pop(chunk)

                # context stage of the PREVIOUS chunk overlaps this chunk's mains
                if prev is not None:
                    emit_context(prev)
                    prev = None

                if chunk < NCHUNKS:
                    # S2: main matmul + tanh (transposed layout [u, t])
                    tanh_sb = tanhp.tile([PART, UC, TCHUNK], mdt, tag="tanh")
                    for uc in range(UC):
                        ps_f = ps_mm.tile([PART, TCHUNK], f32, tag="mm")
                        for dc in range(DC):
                            nc.tensor.matmul(
                                ps_f,
                                w1_sb[:, dc, ts(uc, PART)],
                                ftile_big[:, dc, :],
                                start=(dc == 0),
                                stop=(dc == DC - 1),
                            )
                        nc.scalar.activation(
                            tanh_sb[:, uc, :],
                            ps_f,
                            ACT.Tanh,
                            bias=bias_cols[:, uc, b : b + 1],
                        )
                    prev = {"b": b, "cib": cib, "tanh": tanh_sb, "f_tiles": f_tiles}

    nc.compile()
    _BUILD_CACHE[mm_dt_name] = nc
    return nc


def kernel(**inputs):
    from concourse.bass_utils import run_bass_kernel_spmd

    nc = build_bass()

    feat = np.ascontiguousarray(np.asarray(inputs["features"], dtype=np.float32))
    hid = np.ascontiguousarray(np.asarray(inputs["hidden"], dtype=np.float32))
    shared = {
        k: np.ascontiguousarray(np.asarray(inputs[k], dtype=np.float32))
        for k in ("W1_w", "W1_b", "W2_w", "W2_b", "V_w", "V_b")
    }
    in_maps = []
    for c in range(NCORES):
        m = dict(shared)
        m["features"] = feat[c * BPC : (c + 1) * BPC]
        m["hidden"] = hid[c * BPC : (c + 1) * BPC]
        in_maps.append(m)

    res = run_bass_kernel_spmd(nc, in_maps, list(range(NCORES)))
    return np.concatenate([res.results[c]["context"] for c in range(NCORES)], axis=0)



# revision 2
# speedup vs baseline: 1.0636x; 1.0636x over previous
"""Bahdanau attention kernel for Trainium2 (8 NeuronCores, SPMD data-parallel).

Reference computation (per batch b):
    f_proj = features[b] @ W1_w + W1_b            # [T, U]
    h_proj = hidden[b] @ W2_w + W2_b              # [U]
    score  = tanh(f_proj + h_proj) @ V_w + V_b    # [T]
    attn   = softmax(score)                       # [T]
    context[b] = sum_t attn[t] * features[b, t]   # [D]

Sharding: data-parallel over batch (64 batches / 8 cores = 8 per core),
weights replicated.

Per-core dataflow (bf16 matmul operands, fp32 accumulation / biases):
  - F chunks [128, 4(tile), 512(d)] are cast-DMA'd (SWDGE, fp32->bf16)
    straight from HBM; PE-transposes (bf16) produce F^T [128(d), t] for
    the main matmul; bf16 PSUM->SBUF copies run in DVE 2x mode.
  - main matmul computes f_proj TRANSPOSED: [u(part), t(free)] =
    W1_chunk^T @ F^T in bf16, so the (W1_b + h_proj) bias is a
    per-partition scalar that fuses into the ACT Tanh instruction.
  - score^T [1, t] = V^T @ tanh via M=1 bf16 matmuls; ACT Exp produces
    e = exp(score + V_b) (bf16); row sum via DVE reduction.
    No max-subtraction: |score| <= ||V||_1 + |V_b| ~ 18, safe in fp32.
  - e rows are transposed to columns with tiny identity matmuls;
    context accumulates as e_col^T @ F_native (bf16); final scale by
    1/sum(e) in fp32.
"""

import sys

for _p in ("/opt/trn_rl_repo", "/opt/pypackages"):
    if _p not in sys.path:
        sys.path.insert(0, _p)

import numpy as np

B, T, D, U = 64, 2048, 512, 512
NCORES = 8
BPC = B // NCORES          # batches per core
PART = 128
DC = D // PART             # 4 contraction chunks
UC = U // PART             # 4 u chunks
TCHUNK = 512               # t columns processed per main-matmul group
TILES_PER_CHUNK = TCHUNK // PART          # 4
NCHUNKS = (BPC * T) // TCHUNK             # 32
CHUNKS_PER_BATCH = T // TCHUNK            # 4

_BUILD_CACHE = {}


def build_bass(tag="bf16"):
    """Build + compile the per-core Bass program (same on all cores)."""
    if tag in _BUILD_CACHE:
        return _BUILD_CACHE[tag]

    import concourse.mybir as mybir
    import concourse.tile as tile
    from concourse import bacc
    from concourse.bass import ts
    from concourse.masks import make_identity

    f32 = mybir.dt.float32
    mdt = mybir.dt.float32r   # fp32 (replay) for the small hidden-path
    bdt = mybir.dt.bfloat16   # matmul operand dtype on the hot path
    ACT = mybir.ActivationFunctionType
    AX = mybir.AxisListType

    nc = bacc.Bacc("TRN2", target_bir_lowering=False, debug=False)

    feat = nc.dram_tensor("features", [BPC, T, D], f32, kind="ExternalInput")
    hid = nc.dram_tensor("hidden", [BPC, D], mdt, kind="ExternalInput")
    w1 = nc.dram_tensor("W1_w", [D, U], f32, kind="ExternalInput")
    b1 = nc.dram_tensor("W1_b", [U], f32, kind="ExternalInput")
    w2 = nc.dram_tensor("W2_w", [D, U], mdt, kind="ExternalInput")
    b2 = nc.dram_tensor("W2_b", [U], f32, kind="ExternalInput")
    vw = nc.dram_tensor("V_w", [U, 1], f32, kind="ExternalInput")
    vb = nc.dram_tensor("V_b", [1], f32, kind="ExternalInput")
    out = nc.dram_tensor("context", [BPC, D], f32, kind="ExternalOutput")

    with tile.TileContext(nc) as tc:
        with (
            tc.tile_pool(name="consts", bufs=1) as consts,
            tc.tile_pool(name="fbig", bufs=5) as fbigp,
            tc.tile_pool(name="ftb", bufs=3) as ftb,
            tc.tile_pool(name="tanh", bufs=3) as tanhp,
            tc.tile_pool(name="small", bufs=3) as small,
            tc.tile_pool(name="outp", bufs=2) as outp,
            tc.tile_pool(name="ps_mm", bufs=3, space="PSUM") as ps_mm,
            tc.tile_pool(name="ps_t", bufs=3, space="PSUM") as ps_t,
            tc.tile_pool(name="ps_s", bufs=1, space="PSUM") as ps_s,
            tc.tile_pool(name="ps_c", bufs=1, space="PSUM") as ps_c,
        ):
            # ---------------- constants / setup ----------------
            ident_f32 = consts.tile([PART, PART], f32)
            make_identity(nc, ident_f32)
            ident_r = consts.tile([PART, PART], mdt)
            nc.vector.tensor_copy(ident_r, ident_f32)
            ident_b = consts.tile([PART, PART], bdt)
            nc.vector.tensor_copy(ident_b, ident_f32)

            # F chunks: one SWDGE cast-DMA each (fp32 HBM -> bf16 SBUF)
            fb_tiles = {}

            def load(c):
                if c >= NCHUNKS or c in fb_tiles:
                    return
                b_ = c // CHUNKS_PER_BATCH
                t0 = (c % CHUNKS_PER_BATCH) * TCHUNK
                fb = fbigp.tile(
                    [PART, TILES_PER_CHUNK, D], bdt, tag="F", name=f"fb_{c % 8}"
                )
                nc.gpsimd.dma_start(
                    out=fb,
                    in_=feat.ap()[b_, t0 : t0 + TCHUNK, :].rearrange(
                        "(f p) d -> p f d", p=PART
                    ),
                )
                fb_tiles[c] = fb

            for c in (0, 1, 2):
                load(c)

            # weights: W1/V cast to bf16 in-flight; W2 stays fp32
            w1_sb = consts.tile([PART, DC, U], bdt)
            nc.gpsimd.dma_start(
                out=w1_sb, in_=w1.ap().rearrange("(c p) u -> p c u", p=PART)
            )
            v_sb = consts.tile([PART, UC], bdt)
            nc.gpsimd.dma_start(
                out=v_sb, in_=vw.ap().rearrange("(c p) one -> p (c one)", p=PART)
            )
            w2_sb = consts.tile([PART, DC, U], mdt)
            nc.sync.dma_start(
                out=w2_sb, in_=w2.ap().rearrange("(c p) u -> p c u", p=PART)
            )
            vb_sb = consts.tile([1, 1], f32)
            nc.sync.dma_start(out=vb_sb, in_=vb.ap().rearrange("(one x) -> one x", one=1))

            # W1_b + W2_b as per-partition columns [128, uc]
            b1_sb = consts.tile([PART, UC], f32)
            nc.sync.dma_start(out=b1_sb, in_=b1.ap().rearrange("(c p) -> p c", p=PART))
            b2_sb = consts.tile([PART, UC], f32)
            nc.sync.dma_start(out=b2_sb, in_=b2.ap().rearrange("(c p) -> p c", p=PART))
            b12_sb = consts.tile([PART, UC], f32)
            nc.vector.tensor_add(b12_sb, b1_sb, b2_sb)

            # hidden [BPC, D] -> hiddenT [128(d), dc, BPC]
            hid_sb = consts.tile([BPC, D], mdt)
            nc.sync.dma_start(out=hid_sb, in_=hid.ap())
            hidT_sb = consts.tile([PART, DC, BPC], mdt)
            bias_cols = consts.tile([PART, UC, BPC], f32)

            def emit_setup():
                # emitted after chunk 0's transposes so the PE isn't blocked
                # on the weight/hidden DMAs at kernel start
                for dc in range(DC):
                    ps_h = ps_mm.tile([PART, TCHUNK], mdt, tag="mm", name="ps_h")
                    nc.tensor.transpose(
                        ps_h[:, 0:BPC], hid_sb[:, ts(dc, PART)], ident_r[0:BPC, 0:BPC]
                    )
                    nc.vector.tensor_copy(hidT_sb[:, dc, :], ps_h[:, 0:BPC])
                # h_projT[u, b] = sum_dc W2[dc]^T @ hiddenT[dc]  (+W2_b+W1_b)
                for uc in range(UC):
                    ps_h = ps_mm.tile([PART, TCHUNK], f32, tag="mm", name="ps_h2")
                    for dc in range(DC):
                        nc.tensor.matmul(
                            ps_h[:, 0:BPC],
                            w2_sb[:, dc, ts(uc, PART)],
                            hidT_sb[:, dc, :],
                            start=(dc == 0),
                            stop=(dc == DC - 1),
                        )
                    nc.vector.tensor_scalar_add(
                        bias_cols[:, uc, :], ps_h[:, 0:BPC], b12_sb[:, uc : uc + 1]
                    )

            # ---------------- main loop (epilogue deferred one chunk) ----------------
            prev = None          # chunk state awaiting its score/context stage
            batch_state = {}     # per-batch psum_ctx / running-sum tiles

            def emit_scores(st):
                b, cib = st["b"], st["cib"]
                if cib == 0:
                    batch_state["ps_ctx"] = ps_c.tile([1, D], f32, tag="ctx", name="ps_ctx")
                    batch_state["s_sb"] = small.tile([1, CHUNKS_PER_BATCH], f32, tag="ssum", name="s_sb")
                s_sb = batch_state["s_sb"]

                # score^T [1, t] = V^T @ tanh
                ps_sc = ps_s.tile([1, TCHUNK], f32, tag="score")
                for uc in range(UC):
                    nc.tensor.matmul(
                        ps_sc,
                        v_sb[:, uc : uc + 1],
                        st["tanh"][:, uc, :],
                        start=(uc == 0),
                        stop=(uc == UC - 1),
                    )
                # e = exp(score + V_b); row sum via explicit DVE reduction
                e_sb = small.tile([1, TCHUNK], bdt, tag="e_sb")
                nc.scalar.activation(e_sb, ps_sc, ACT.Exp, bias=vb_sb)
                nc.vector.reduce_sum(s_sb[:, cib : cib + 1], e_sb, axis=AX.X)
                st["e_sb"] = e_sb

            def emit_context(st):
                b, cib = st["b"], st["cib"]
                ps_ctx = batch_state["ps_ctx"]
                s_sb = batch_state["s_sb"]
                e_sb = st["e_sb"]
                # transpose e row -> columns via [1,0]-identity-row matmuls
                ps_e = ps_s.tile([PART, 2 * TILES_PER_CHUNK], f32, tag="score", name="ps_e")
                for j in range(TILES_PER_CHUNK):
                    nc.tensor.matmul(
                        ps_e[:, 2 * j : 2 * j + 2],
                        e_sb[0:1, ts(j, PART)],
                        ident_b[0:1, 0:2],
                        start=True,
                        stop=True,
                    )
                e_col = small.tile([PART, TILES_PER_CHUNK], bdt, tag="e_col")
                nc.vector.tensor_copy(
                    e_col, ps_e.rearrange("p (j two) -> p two j", two=2)[:, 0, :]
                )
                # context accumulation: ps_ctx [1, D] += e_col_j^T @ F_j
                fb = st["fb"]
                for j in range(TILES_PER_CHUNK):
                    nc.tensor.matmul(
                        ps_ctx,
                        e_col[:, j : j + 1],
                        fb[:, j, :],
                        start=(cib == 0 and j == 0),
                        stop=(cib == CHUNKS_PER_BATCH - 1 and j == TILES_PER_CHUNK - 1),
                    )
                if cib == CHUNKS_PER_BATCH - 1:
                    ssum = small.tile([1, 1], f32, tag="ssum1")
                    nc.vector.reduce_sum(ssum, s_sb, axis=AX.X)
                    rec = small.tile([1, 1], f32, tag="rec")
                    nc.vector.reciprocal(rec, ssum)
                    ctx_sb = outp.tile([1, D], f32, tag="ctx_sb")
                    nc.vector.tensor_scalar_mul(ctx_sb, ps_ctx, rec)
                    nc.sync.dma_start(out=out.ap()[b : b + 1, :], in_=ctx_sb)

            s1_state = {}

            def emit_s1(c):
                # PE-transpose chunk c's F tiles (bf16) into F^T layout
                fb = fb_tiles[c]
                ftile_big = ftb.tile([PART, DC, TCHUNK], bdt, tag="FT", name="ftile_big")
                for j in range(TILES_PER_CHUNK):
                    ps_tr = ps_t.tile([PART, TCHUNK], bdt, tag="T", name="ps_tr")
                    for dc in range(DC):
                        nc.tensor.transpose(
                            ps_tr[:, ts(dc, PART)], fb[:, j, ts(dc, PART)], ident_b
                        )
                    nc.vector.tensor_copy(
                        ftile_big[:, :, ts(j, PART)],
                        ps_tr.rearrange("p (c t) -> p c t", c=DC),
                    )
                s1_state[c] = (fb, ftile_big)

            emit_s1(0)

            for chunk in range(NCHUNKS + 1):
                # V-dot + exp of the previous chunk lead this chunk
                if prev is not None:
                    emit_scores(prev)

                # transpose stage one chunk ahead, so this chunk's mains
                # never wait on the F^T copies
                if chunk + 1 < NCHUNKS:
                    emit_s1(chunk + 1)

                # keep the cast-DMA runway ~3 chunks deep
                load(chunk + 3)

                if chunk == 0:
                    emit_setup()

                if chunk < NCHUNKS:
                    b = chunk // CHUNKS_PER_BATCH
                    cib = chunk % CHUNKS_PER_BATCH
                    fb, ftile_big = s1_state.pop(chunk)

                # context stage of the PREVIOUS chunk overlaps this chunk's mains
                if prev is not None:
                    emit_context(prev)
                    prev = None

                if chunk < NCHUNKS:
                    # S2: main matmul + tanh (transposed layout [u, t])
                    tanh_sb = tanhp.tile([PART, UC, TCHUNK], bdt, tag="tanh")
                    for uc in range(UC):
                        ps_f = ps_mm.tile([PART, TCHUNK], f32, tag="mm")
                        for dc in range(DC):
                            nc.tensor.matmul(
                                ps_f,
                                w1_sb[:, dc, ts(uc, PART)],
                                ftile_big[:, dc, :],
                                start=(dc == 0),
                                stop=(dc == DC - 1),
                            )
                        nc.scalar.activation(
                            tanh_sb[:, uc, :],
                            ps_f,
                            ACT.Tanh,
                            bias=bias_cols[:, uc, b : b + 1],
                        )
                    prev = {"b": b, "cib": cib, "tanh": tanh_sb, "fb": fb}

    nc.compile()
    _BUILD_CACHE[tag] = nc
    return nc


def kernel(**inputs):
    from concourse.bass_utils import run_bass_kernel_spmd

    nc = build_bass()

    feat = np.ascontiguousarray(np.asarray(inputs["features"], dtype=np.float32))
    hid = np.ascontiguousarray(np.asarray(inputs["hidden"], dtype=np.float32))
    shared = {
        k: np.ascontiguousarray(np.asarray(inputs[k], dtype=np.float32))
        for k in ("W1_w", "W1_b", "W2_w", "W2_b", "V_w", "V_b")
    }
    in_maps = []
    for c in range(NCORES):
        m = dict(shared)
        m["features"] = feat[c * BPC : (c + 1) * BPC]
        m["hidden"] = hid[c * BPC : (c + 1) * BPC]
        in_maps.append(m)

    res = run_bass_kernel_spmd(nc, in_maps, list(range(NCORES)))
    return np.concatenate([res.results[c]["context"] for c in range(NCORES)], axis=0)


# revision 10
# speedup vs baseline: 1.1942x; 1.1228x over previous
"""Bahdanau attention kernel for Trainium2 (8 NeuronCores, SPMD data-parallel).

Reference computation (per batch b):
    f_proj = features[b] @ W1_w + W1_b            # [T, U]
    h_proj = hidden[b] @ W2_w + W2_b              # [U]
    score  = tanh(f_proj + h_proj) @ V_w + V_b    # [T]
    attn   = softmax(score)                       # [T]
    context[b] = sum_t attn[t] * features[b, t]   # [D]

Sharding: data-parallel over batch (64 batches / 8 cores = 8 per core),
weights replicated.

Per-core dataflow (bf16 matmul operands, fp32 accumulation / biases):
  - F chunks [128, 4(tile), 512(d)] are cast-DMA'd (SWDGE, fp32->bf16)
    straight from HBM; PE-transposes (bf16) produce F^T [128(d), t] for
    the main matmul; bf16 PSUM->SBUF copies run in DVE 2x mode.
  - main matmul computes f_proj TRANSPOSED: [u(part), t(free)] =
    W1_chunk^T @ F^T in bf16, so the (W1_b + h_proj) bias is a
    per-partition scalar that fuses into the ACT Tanh instruction.
  - score/context stages are deferred TWO chunks so tanh latency never
    stalls the PE, and their M=1 matmuls are column-group tiled
    (tile_position) so the four per-chunk matmuls execute concurrently
    in disjoint PE quadrants; the 4 partial rows are folded with a k=4
    ones-matmul (score: per chunk; context: once per batch).
  - e = exp(score + V_b) (bf16); row sum via DVE; no max-subtraction:
    |score| <= ||V||_1 + |V_b| ~ 18, safe in fp32.
"""

import sys

for _p in ("/opt/trn_rl_repo", "/opt/pypackages"):
    if _p not in sys.path:
        sys.path.insert(0, _p)

import numpy as np

B, T, D, U = 64, 2048, 512, 512
NCORES = 8
BPC = B // NCORES          # batches per core
PART = 128
DC = D // PART             # 4 contraction chunks
UC = U // PART             # 4 u chunks
TCHUNK = 512               # t columns processed per main-matmul group
TILES_PER_CHUNK = TCHUNK // PART          # 4
NCHUNKS = (BPC * T) // TCHUNK             # 32
CHUNKS_PER_BATCH = T // TCHUNK            # 4

_BUILD_CACHE = {}


def build_bass(tag="v3"):
    """Build + compile the per-core Bass program (same on all cores)."""
    if tag in _BUILD_CACHE:
        return _BUILD_CACHE[tag]

    import concourse.mybir as mybir
    import concourse.tile as tile
    from concourse import bacc
    from concourse.bass import ts
    from concourse.masks import make_identity

    f32 = mybir.dt.float32
    mdt = mybir.dt.float32r   # fp32 (replay) for fold / hidden-path matmuls
    bdt = mybir.dt.bfloat16   # matmul operand dtype on the hot path
    ACT = mybir.ActivationFunctionType
    AX = mybir.AxisListType

    nc = bacc.Bacc("TRN2", target_bir_lowering=False, debug=False)

    feat = nc.dram_tensor("features", [BPC, T, D], f32, kind="ExternalInput")
    hid = nc.dram_tensor("hidden", [BPC, D], mdt, kind="ExternalInput")
    w1 = nc.dram_tensor("W1_w", [D, U], f32, kind="ExternalInput")
    b1 = nc.dram_tensor("W1_b", [U], f32, kind="ExternalInput")
    w2 = nc.dram_tensor("W2_w", [D, U], mdt, kind="ExternalInput")
    b2 = nc.dram_tensor("W2_b", [U], f32, kind="ExternalInput")
    vw = nc.dram_tensor("V_w", [U, 1], f32, kind="ExternalInput")
    vb = nc.dram_tensor("V_b", [1], f32, kind="ExternalInput")
    out = nc.dram_tensor("context", [BPC, D], f32, kind="ExternalOutput")

    with tile.TileContext(nc) as tc:
        with (
            tc.tile_pool(name="consts", bufs=1) as consts,
            tc.tile_pool(name="fbig", bufs=6) as fbigp,
            tc.tile_pool(name="ftb", bufs=3) as ftb,
            tc.tile_pool(name="tanh", bufs=3) as tanhp,
            tc.tile_pool(name="small", bufs=3) as small,
            tc.tile_pool(name="outp", bufs=2) as outp,
            tc.tile_pool(name="ps_mm", bufs=2, space="PSUM") as ps_mm,
            tc.tile_pool(name="ps_t", bufs=3, space="PSUM") as ps_t,
            tc.tile_pool(name="ps_s", bufs=1, space="PSUM") as ps_s,
            tc.tile_pool(name="ps_c", bufs=1, space="PSUM") as ps_c,
        ):
            # ---------------- constants / setup ----------------
            ident_f32 = consts.tile([PART, PART], f32)
            make_identity(nc, ident_f32)
            ident_r = consts.tile([PART, PART], mdt)
            nc.vector.tensor_copy(ident_r, ident_f32)
            ident_b = consts.tile([PART, PART], bdt)
            nc.vector.tensor_copy(ident_b, ident_f32)
            ones_f = consts.tile([PART, 1], f32)
            nc.gpsimd.memset(ones_f, 1.0)
            ones1 = consts.tile([PART, 1], mdt)
            nc.vector.tensor_copy(ones1, ones_f)

            # F chunks: one SWDGE cast-DMA each (fp32 HBM -> bf16 SBUF)
            fb_tiles = {}

            def load(c):
                if c >= NCHUNKS or c in fb_tiles:
                    return
                b_ = c // CHUNKS_PER_BATCH
                t0 = (c % CHUNKS_PER_BATCH) * TCHUNK
                fb = fbigp.tile(
                    [PART, TILES_PER_CHUNK, D], bdt, tag="F", name=f"fb_{c % 8}"
                )
                nc.gpsimd.dma_start(
                    out=fb,
                    in_=feat.ap()[b_, t0 : t0 + TCHUNK, :].rearrange(
                        "(f p) d -> p f d", p=PART
                    ),
                )
                fb_tiles[c] = fb

            for c in (0, 1, 2):
                load(c)

            # weights: W1/V cast to bf16 in-flight; W2 stays fp32
            w1_sb = consts.tile([PART, DC, U], bdt)
            nc.gpsimd.dma_start(
                out=w1_sb, in_=w1.ap().rearrange("(c p) u -> p c u", p=PART)
            )
            v_sb = consts.tile([PART, UC], bdt)
            nc.gpsimd.dma_start(
                out=v_sb, in_=vw.ap().rearrange("(c p) one -> p (c one)", p=PART)
            )
            # V padded to M=32 (column 0 = V, rest zero) so the col-group
            # tiled score matmuls write contiguous partition ranges
            vz = consts.tile([PART, UC * 32], f32)
            nc.gpsimd.memset(vz, 0.0)
            v32 = consts.tile([PART, UC, 32], bdt)
            nc.vector.tensor_copy(v32, vz.rearrange("p (c k) -> p c k", k=32))
            for uc in range(UC):
                nc.vector.tensor_copy(v32[:, uc, 0:1], v_sb[:, uc : uc + 1])
            w2_sb = consts.tile([PART, DC, U], mdt)
            nc.sync.dma_start(
                out=w2_sb, in_=w2.ap().rearrange("(c p) u -> p c u", p=PART)
            )
            vb_sb = consts.tile([1, 1], f32)
            nc.sync.dma_start(out=vb_sb, in_=vb.ap().rearrange("(one x) -> one x", one=1))

            # W1_b + W2_b as per-partition columns [128, uc]
            b1_sb = consts.tile([PART, UC], f32)
            nc.sync.dma_start(out=b1_sb, in_=b1.ap().rearrange("(c p) -> p c", p=PART))
            b2_sb = consts.tile([PART, UC], f32)
            nc.sync.dma_start(out=b2_sb, in_=b2.ap().rearrange("(c p) -> p c", p=PART))
            b12_sb = consts.tile([PART, UC], f32)
            nc.vector.tensor_add(b12_sb, b1_sb, b2_sb)

            # hidden [BPC, D] -> hiddenT [128(d), dc, BPC]
            hid_sb = consts.tile([BPC, D], mdt)
            nc.sync.dma_start(out=hid_sb, in_=hid.ap())
            hidT_sb = consts.tile([PART, DC, BPC], mdt)
            bias_cols = consts.tile([PART, UC, BPC], f32)

            def emit_setup():
                # emitted after chunk 0's transposes so the PE isn't blocked
                # on the weight/hidden DMAs at kernel start
                for dc in range(DC):
                    ps_h = ps_mm.tile([PART, TCHUNK], mdt, tag="mm", name="ps_h")
                    nc.tensor.transpose(
                        ps_h[:, 0:BPC], hid_sb[:, ts(dc, PART)], ident_r[0:BPC, 0:BPC]
                    )
                    nc.vector.tensor_copy(hidT_sb[:, dc, :], ps_h[:, 0:BPC])
                # h_projT[u, b] = sum_dc W2[dc]^T @ hiddenT[dc]  (+W2_b+W1_b)
                for uc in range(UC):
                    ps_h = ps_mm.tile([PART, TCHUNK], f32, tag="mm", name="ps_h2")
                    for dc in range(DC):
                        nc.tensor.matmul(
                            ps_h[:, 0:BPC],
                            w2_sb[:, dc, ts(uc, PART)],
                            hidT_sb[:, dc, :],
                            start=(dc == 0),
                            stop=(dc == DC - 1),
                        )
                    nc.vector.tensor_scalar_add(
                        bias_cols[:, uc, :], ps_h[:, 0:BPC], b12_sb[:, uc : uc + 1]
                    )

            # ---------------- main loop (epilogue deferred TWO chunks) -------------
            pending = []         # chunk states awaiting their score/context stage
            batch_state = {}     # per-batch psum ctx-partials / running-sum tiles

            def emit_scores(st):
                b, cib = st["b"], st["cib"]
                if cib == 0:
                    batch_state["ps_cpar"] = ps_c.tile([PART, D], f32, tag="ctx", name="ps_cpar")
                    batch_state["s_sb"] = small.tile([1, CHUNKS_PER_BATCH], f32, tag="ssum", name="s_sb")
                s_sb = batch_state["s_sb"]

                # 4 col-group-tiled M=32 matmuls (zero-padded V) -> partial
                # scores at partitions {0,32,64,96}, zeros elsewhere; fold
                # all 128 partitions with a ones-matmul
                ps_par = ps_s.tile([PART, TCHUNK], f32, tag="par", name="ps_par")
                for uc in range(UC):
                    nc.tensor.matmul(
                        ps_par[32 * uc : 32 * uc + 32, :],
                        v32[:, uc, :],
                        st["tanh"][:, uc, :],
                        start=True,
                        stop=True,
                        tile_position=(0, 32 * uc),
                    )
                par_sb = small.tile([PART, TCHUNK], mdt, tag="psb", name="par_sb", bufs=2)
                nc.vector.tensor_copy(par_sb, ps_par)
                ps_sc = ps_s.tile([1, TCHUNK], f32, tag="score", name="ps_sc")
                nc.tensor.matmul(ps_sc, ones1, par_sb, start=True, stop=True)

                # e = exp(score + V_b); row sum via explicit DVE reduction
                e_sb = small.tile([1, TCHUNK], bdt, tag="e_sb")
                nc.scalar.activation(e_sb, ps_sc, ACT.Exp, bias=vb_sb)
                nc.vector.reduce_sum(s_sb[:, cib : cib + 1], e_sb, axis=AX.X)
                st["e_sb"] = e_sb

            def emit_context(st):
                b, cib = st["b"], st["cib"]
                ps_cpar = batch_state["ps_cpar"]
                s_sb = batch_state["s_sb"]
                e_sb = st["e_sb"]
                # transpose e row -> 32-wide column blocks (col 0 = e, rest 0)
                ps_e = ps_s.tile([PART, 32 * TILES_PER_CHUNK], f32, tag="score", name="ps_e")
                for j in range(TILES_PER_CHUNK):
                    nc.tensor.matmul(
                        ps_e[:, 32 * j : 32 * j + 32],
                        e_sb[0:1, ts(j, PART)],
                        ident_b[0:1, 0:32],
                        start=True,
                        stop=True,
                    )
                e_col = small.tile([PART, TILES_PER_CHUNK, 32], bdt, tag="e_col")
                nc.vector.tensor_copy(
                    e_col, ps_e.rearrange("p (j c) -> p j c", c=32)
                )
                # context partials: 4 col-group-tiled M=32 matmuls, one
                # accumulation group per j, folded once per batch
                fb = st["fb"]
                for j in range(TILES_PER_CHUNK):
                    nc.tensor.matmul(
                        ps_cpar[32 * j : 32 * j + 32, :],
                        e_col[:, j, :],
                        fb[:, j, :],
                        start=(cib == 0),
                        stop=(cib == CHUNKS_PER_BATCH - 1),
                        tile_position=(0, 32 * j),
                        skip_group_check=True,
                    )
                if cib == CHUNKS_PER_BATCH - 1:
                    cpar_sb = small.tile([PART, D], mdt, tag="cpsb", name="cpar_sb", bufs=2)
                    nc.vector.tensor_copy(cpar_sb, ps_cpar)
                    ps_fin = ps_s.tile([1, D], f32, tag="par", name="ps_fin")
                    nc.tensor.matmul(ps_fin, ones1, cpar_sb, start=True, stop=True)
                    ssum = small.tile([1, 1], f32, tag="ssum1")
                    nc.vector.reduce_sum(ssum, s_sb, axis=AX.X)
                    rec = small.tile([1, 1], f32, tag="rec")
                    nc.vector.reciprocal(rec, ssum)
                    ctx_sb = outp.tile([1, D], f32, tag="ctx_sb")
                    nc.vector.tensor_scalar_mul(ctx_sb, ps_fin, rec)
                    nc.sync.dma_start(out=out.ap()[b : b + 1, :], in_=ctx_sb)

            s1_state = {}

            def emit_s1(c):
                # PE-transpose chunk c's F tiles (bf16) into F^T layout
                fb = fb_tiles[c]
                ftile_big = ftb.tile([PART, DC, TCHUNK], bdt, tag="FT", name="ftile_big")
                for j in range(TILES_PER_CHUNK):
                    ps_tr = ps_t.tile([PART, TCHUNK], bdt, tag="T", name="ps_tr")
                    for dc in range(DC):
                        nc.tensor.transpose(
                            ps_tr[:, ts(dc, PART)], fb[:, j, ts(dc, PART)], ident_b
                        )
                    nc.vector.tensor_copy(
                        ftile_big[:, :, ts(j, PART)],
                        ps_tr.rearrange("p (c t) -> p c t", c=DC),
                    )
                s1_state[c] = (fb, ftile_big)

            emit_s1(0)

            for chunk in range(NCHUNKS + 2):
                # score stage of chunk-2 leads this chunk (tanh long done)
                st = None
                if len(pending) == 2 or (chunk >= NCHUNKS and pending):
                    st = pending.pop(0)
                    emit_scores(st)

                # transpose stage one chunk ahead, so this chunk's mains
                # never wait on the F^T copies
                if chunk + 1 < NCHUNKS:
                    emit_s1(chunk + 1)

                # keep the cast-DMA runway ~3 chunks deep
                load(chunk + 3)

                if chunk == 0:
                    emit_setup()

                # context stage of chunk-2 overlaps this chunk's mains
                if st is not None:
                    emit_context(st)

                if chunk < NCHUNKS:
                    b = chunk // CHUNKS_PER_BATCH
                    cib = chunk % CHUNKS_PER_BATCH
                    fb, ftile_big = s1_state.pop(chunk)

                    # S2: main matmul + tanh (transposed layout [u, t])
                    tanh_sb = tanhp.tile([PART, UC, TCHUNK], bdt, tag="tanh")
                    for uc in range(UC):
                        ps_f = ps_mm.tile([PART, TCHUNK], f32, tag="mm")
                        for dc in range(DC):
                            nc.tensor.matmul(
                                ps_f,
                                w1_sb[:, dc, ts(uc, PART)],
                                ftile_big[:, dc, :],
                                start=(dc == 0),
                                stop=(dc == DC - 1),
                            )
                        nc.scalar.activation(
                            tanh_sb[:, uc, :],
                            ps_f,
                            ACT.Tanh,
                            bias=bias_cols[:, uc, b : b + 1],
                        )
                    pending.append({"b": b, "cib": cib, "tanh": tanh_sb, "fb": fb})

    nc.compile()
    _BUILD_CACHE[tag] = nc
    return nc


def kernel(**inputs):
    from concourse.bass_utils import run_bass_kernel_spmd

    nc = build_bass()

    feat = np.ascontiguousarray(np.asarray(inputs["features"], dtype=np.float32))
    hid = np.ascontiguousarray(np.asarray(inputs["hidden"], dtype=np.float32))
    shared = {
        k: np.ascontiguousarray(np.asarray(inputs[k], dtype=np.float32))
        for k in ("W1_w", "W1_b", "W2_w", "W2_b", "V_w", "V_b")
    }
    in_maps = []
    for c in range(NCORES):
        m = dict(shared)
        m["features"] = feat[c * BPC : (c + 1) * BPC]
        m["hidden"] = hid[c * BPC : (c + 1) * BPC]
        in_maps.append(m)

    res = run_bass_kernel_spmd(nc, in_maps, list(range(NCORES)))
    return np.concatenate([res.results[c]["context"] for c in range(NCORES)], axis=0)
